# revision 18
# baseline (speedup 1.0000x reference)
"""Trainium2 Bass kernel for nn_AttentionBlock (B=8, T=2048, C=512).

Data-parallel over batch: one batch element per NeuronCore (8 cores).

All matmuls are fp8e4 (e4m3) MatmulPerfMode.DoubleRow: the PE virtualizes
to 128x256, so a 512-deep contraction takes 2 instructions at ~174 ns
(HW-measured) vs ~323 ns for fp32r.

Score algebra: softmax is over the QUERY axis (reference quirk), which is
the FREE axis of the transposed score layout st[k, q]. Any per-k additive
term cancels between exp numerator and row-sum denominator, so with
  st[k,q] = (x Wk^T + bk)[k] . (x Wq^T + bq)[q]
the bq-dependent terms (per-k) cancel, and
  st'[k,q] = yhat[k] . x[q],   yhat = x (Wk^T Wq) + bk Wq
yields the exact same softmax. This removes the whole q-projection, the
Wq/Wk PE transposes, and one PSUM-evacuation pass.

Per-core pipeline (x: [T, C]):
  xT8[c, t]  = fp8(x^T)                      bf16 PE transpose + cast
  M8         = fp8(Wk^T Wq)  [c, c']         from NATURAL weight loads
  w_sb[c']   = bk @ Wq                       8 tiny DR matmuls
  yT8[c', t] = fp8(M^T x^T + w_sb)           "k-side" projection
  vbf[t, d]  = bf16(x @ Wv^T + bv)
  st[k, q]   = sum_c' yT8[c',k] xT8[c',q]    DR pairs over c'
  e8[k, q]   = fp8(exp(st/sqrt(D)))          causal-masked, fp32 sums S
  v8[k, :]   = fp8(vbf[k, :] * 64/S[k])      on GpSimd; 64 = fp8 range shift
  out[q, :]  = (1/64) sum_k e8[k,q] v8[k,:]  DR pairs over k-chunks
  output = concat([x, out], axis=-1)
"""

import numpy as np

import concourse.bass as bass
import concourse.mybir as mybir
import concourse.tile as tile
from concourse import bacc

B, T, C = 8, 2048, 512
D = 512                      # KEY_SIZE == VALUE_SIZE == 512
P = 128                      # partitions
NT = T // P                  # 16 t-chunks
NC4 = C // P                 # 4 contraction chunks
ND = D // P                  # 4 d-chunks
QS = 512                     # q-slice width for score matmuls
NQ = T // QS                 # 4 q-slices
SCALE = float(1.0 / np.sqrt(D))
NEG = -1.0e30
ESC = 64.0                   # fp8 range shift for attention weights

F32 = mybir.dt.float32
BF16 = mybir.dt.bfloat16
FP8 = mybir.dt.float8e4
DR = mybir.MatmulPerfMode.DoubleRow

# max_phase: 0=loads+transposes+M, 1=+projections, 2=+score MMs (no exp),
# 3=+softmax (exp/S/v8), 4=full (AV+out). For HW phase-isolation timing.
CFG = {"copy_mod": 2, "max_phase": 4}

MM_DTYPES = {"f32r": mybir.dt.float32r}  # vestigial (hw_time compat)


def build_nc(mm_dtype="f32r"):
    nc = bacc.Bacc(trn_type="TRN2", target_bir_lowering=False)

    x = nc.dram_tensor("x", [T, C], F32, kind="ExternalInput").ap()
    Wq = nc.dram_tensor("Wq", [D, C], F32, kind="ExternalInput").ap()
    bq = nc.dram_tensor("bq", [D], F32, kind="ExternalInput").ap()
    Wk = nc.dram_tensor("Wk", [D, C], F32, kind="ExternalInput").ap()
    bk = nc.dram_tensor("bk", [D], F32, kind="ExternalInput").ap()
    Wv = nc.dram_tensor("Wv", [D, C], F32, kind="ExternalInput").ap()
    bv = nc.dram_tensor("bv", [D], F32, kind="ExternalInput").ap()
    out = nc.dram_tensor("out", [T, 2 * C], F32, kind="ExternalOutput").ap()

    with tile.TileContext(nc) as tc:
        _emit(nc, tc, x, (Wq, bq), (Wk, bk), (Wv, bv), out, None)
    nc.compile()
    return nc


def _emit(nc, tc, x, wq, wk, wv, out, _mdt):
    from contextlib import ExitStack

    Wq, _bq = wq   # bq cancels in the q-axis softmax; unused
    Wk, bk = wk
    Wv, bv = wv

    _ncopy = [0]

    def copy_ps(dst, src, bias=None, eng=None):
        """PSUM->SBUF copy alternating DVE/ACT, optional per-partition bias."""
        if eng is None:
            _ncopy[0] += 1
            eng = "act" if (_ncopy[0] % CFG["copy_mod"] == 0) else "dve"
        if eng == "dve":
            if bias is None:
                nc.vector.tensor_copy(dst, src)
            else:
                nc.vector.tensor_scalar_add(out=dst, in0=src, scalar1=bias)
        else:
            if bias is None:
                nc.scalar.activation(
                    out=dst, in_=src, func=mybir.ActivationFunctionType.Identity
                )
            else:
                nc.scalar.activation(
                    out=dst, in_=src,
                    func=mybir.ActivationFunctionType.Identity, bias=bias,
                )

    with ExitStack() as ctx:
        const = ctx.enter_context(tc.tile_pool(name="const", bufs=1))
        persist = ctx.enter_context(tc.tile_pool(name="persist", bufs=1))
        stats = ctx.enter_context(tc.tile_pool(name="stats", bufs=4))
        outsb = ctx.enter_context(tc.tile_pool(name="outsb", bufs=3))
        psum_acc = ctx.enter_context(
            tc.tile_pool(name="psum_acc", bufs=4, space="PSUM")
        )
        psum_st = ctx.enter_context(
            tc.tile_pool(name="psum_st", bufs=4, space="PSUM")
        )

        # ---- constants ----
        identf = const.tile([P, P], F32, name="identf")
        nc.gpsimd.memset(identf, 0.0)
        nc.gpsimd.affine_select(
            out=identf, in_=identf, compare_op=mybir.AluOpType.not_equal,
            fill=1.0, base=0, pattern=[[-1, P]], channel_multiplier=1,
        )
        identb = const.tile([P, P], BF16, name="identb")
        nc.vector.tensor_copy(identb, identf)
        # tri[p, j] = 0 where j >= p (valid), NEG where j < p (masked)
        tri = const.tile([P, P], F32, name="tri")
        nc.gpsimd.memset(tri, 0.0)
        nc.gpsimd.affine_select(
            out=tri, in_=tri, compare_op=mybir.AluOpType.is_ge,
            fill=NEG, base=0, pattern=[[1, P]], channel_multiplier=-1,
        )
        # bv broadcast to all partitions via rank-1 fp32 matmul
        ones_f = const.tile([1, P], F32, name="ones_f")
        nc.gpsimd.memset(ones_f, 1.0)
        bv_f = const.tile([1, D], F32, name="bv_f")
        nc.gpsimd.dma_start(out=bv_f, in_=bv.unsqueeze(0))
        bv_full = const.tile([P, D], F32, name="bv_full")
        ps_bv = psum_acc.tile([P, D], F32, name="ps_bv", tag="acc")
        nc.tensor.matmul(ps_bv, ones_f, bv_f, start=True, stop=True)
        nc.vector.tensor_copy(bv_full, ps_bv)

        # bk as fp8 [128, 4, 1] for the w_sb = bk @ Wq matmuls
        bk_sb = const.tile([P, ND], F32, name="bk_sb")
        for dc in range(ND):
            nc.gpsimd.dma_start(
                out=bk_sb[:, dc : dc + 1],
                in_=bk[dc * P : (dc + 1) * P].unsqueeze(-1),
            )
        bk8 = const.tile([P, ND, 1], FP8, name="bk8")
        nc.vector.tensor_copy(bk8[:, :, 0], bk_sb)

        # ---- persistent fp8/bf16 activations ----
        xT8 = persist.tile([P, NQ, NC4, QS], FP8, name="xT8", tag="xT8")
        yT8 = persist.tile([P, NT, NC4, P], FP8, name="yT8", tag="yT8")
        e8 = persist.tile([P, NT, NT, P], FP8, name="e8", tag="e8")
        vbf = persist.tile([P, NT, D], BF16, name="vbf", tag="vbf")
        v8 = persist.tile([P, NT, D], FP8, name="v8", tag="v8")
        Wq8 = persist.tile([P, ND, C], FP8, name="Wq8", tag="Wq8")
        Wk8 = persist.tile([P, ND, C], FP8, name="Wk8", tag="Wk8")
        M8 = persist.tile([P, NC4, NC4, P], FP8, name="M8", tag="M8")
        wvT8 = persist.tile([P, NC4, D], FP8, name="wvT8", tag="wvT8")
        w_sb = const.tile([P, NC4], F32, name="w_sb")

        # zero the 8 pair-diagonal blocks of e8 that AV reads but exp never
        # writes: e8[:, 2j+1, block 2j]
        for j in range(NT // 2):
            kc = 2 * j + 1
            nc.vector.memset(e8[:, 2 * j, kc, :], 0.0)

        # ---- phase 0: loads, F32 PE transposes (cast at PSUM copy), M ----
        with tc.tile_pool(name="loads", bufs=1) as loads:
            # x loads first (the long pole): 8 batched DMAs of 2 chunks each
            xn_all = []
            for g in range(8):
                xn = loads.tile([P, 2, C], F32, name=f"xn{g}",
                                tag=f"xn{g % 4}", bufs=2)
                nc.sync.dma_start(
                    out=xn,
                    in_=x[g * 2 * P : (g + 1) * 2 * P, :].rearrange(
                        "(a p) c -> p a c", p=P
                    ),
                )
                xb = loads.tile([P, 2, C], BF16, name=f"xb{g}",
                                tag=f"xb{g % 4}", bufs=2)
                if g % 2:
                    nc.scalar.activation(
                        out=xb, in_=xn,
                        func=mybir.ActivationFunctionType.Identity,
                    )
                else:
                    nc.vector.tensor_copy(xb, xn)
                xn_all.append(xb)

            # Wq/Wk natural loads -> fp8 via DVE cast (no transposes needed)
            for W8, W, wt in ((Wk8, Wk, "wk"), (Wq8, Wq, "wq")):
                for h in range(2):
                    wn = loads.tile([P, 2, C], F32, name=f"{wt}{h}",
                                    tag=f"{wt}{h}", bufs=1)
                    nc.sync.dma_start(
                        out=wn,
                        in_=W[h * 2 * P : (h + 1) * 2 * P, :].rearrange(
                            "(a p) c -> p a c", p=P
                        ),
                    )
                    for j in range(2):
                        nc.vector.tensor_copy(W8[:, h * 2 + j, :], wn[:, j, :])

            # M~ = Wk^T Wq  [c, c']  (DR pairs over d)
            for c1 in range(NC4):
                ps = psum_acc.tile([P, C], F32, name="ps_m", tag="acc")
                for s in range(2):
                    nc.tensor.matmul(
                        ps,
                        Wk8[:, 2 * s : 2 * s + 2, c1 * P : (c1 + 1) * P],
                        Wq8[:, 2 * s : 2 * s + 2, :],
                        start=(s == 0),
                        stop=(s == 1),
                        perf_mode=DR,
                    )
                copy_ps(M8[:, :, c1, :], ps)

            # w_sb[c'] = bk @ Wq   (tiny N=1 DR matmuls per c'-block)
            for c1 in range(NC4):
                ps = psum_acc.tile([P, 1], F32, name="ps_w", tag="acc")
                for s in range(2):
                    nc.tensor.matmul(
                        ps,
                        Wq8[:, 2 * s : 2 * s + 2, c1 * P : (c1 + 1) * P],
                        bk8[:, 2 * s : 2 * s + 2, :],
                        start=(s == 0),
                        stop=(s == 1),
                        perf_mode=DR,
                    )
                nc.vector.tensor_copy(w_sb[:, c1 : c1 + 1], ps)

            def x_group(tg):
                xsl = [xn_all[tg * 2][:, 0, :], xn_all[tg * 2][:, 1, :],
                       xn_all[tg * 2 + 1][:, 0, :], xn_all[tg * 2 + 1][:, 1, :]]
                for cc in range(NC4):
                    ps = psum_st.tile([P, D], BF16, name="ps_xt", tag="st")
                    for j in range(4):
                        nc.tensor.transpose(
                            ps[:, j * P : (j + 1) * P],
                            xsl[j][:, cc * P : (cc + 1) * P],
                            identb,
                        )
                    copy_ps(xT8[:, tg, cc, :], ps)

            for tg in range(4):
                x_group(tg)

            # Wv load -> F32 PE transpose -> fp8 at copy
            wvn = []
            for h in range(2):
                wn = loads.tile([P, 2, C], F32, name=f"wvn{h}",
                                tag=f"wvn{h}", bufs=1)
                nc.sync.dma_start(
                    out=wn,
                    in_=Wv[h * 2 * P : (h + 1) * 2 * P, :].rearrange(
                        "(a p) c -> p a c", p=P
                    ),
                )
                wb = loads.tile([P, 2, C], BF16, name=f"wvb{h}",
                                tag=f"wvb{h}", bufs=1)
                nc.vector.tensor_copy(wb, wn)
                wvn.append(wb)
            wsl = [wvn[0][:, 0, :], wvn[0][:, 1, :],
                   wvn[1][:, 0, :], wvn[1][:, 1, :]]
            for cc in range(NC4):
                ps = psum_acc.tile([P, D], BF16, name="ps_wt", tag="acc")
                for dc in range(ND):
                    nc.tensor.transpose(
                        ps[:, dc * P : (dc + 1) * P],
                        wsl[dc][:, cc * P : (cc + 1) * P],
                        identb,
                    )
                copy_ps(wvT8[:, cc, :], ps, eng="act")

            if CFG["max_phase"] < 1:
                return
            # ---- phase 1: projections (fp8 DoubleRow) ----
            # yhat^T = M^T x^T + w_sb  (the bias-folded "k-side")
            for dc in range(NC4):
                for qs in range(NQ):
                    ps = psum_acc.tile([P, QS], F32, name="ps_y", tag="acc")
                    for s in range(2):
                        nc.tensor.matmul(
                            ps,
                            M8[:, dc, 2 * s : 2 * s + 2, :],
                            xT8[:, qs, 2 * s : 2 * s + 2, :],
                            start=(s == 0),
                            stop=(s == 1),
                            perf_mode=DR,
                        )
                    copy_ps(
                        yT8[:, qs * 4 : (qs + 1) * 4, dc, :], ps,
                        bias=w_sb[:, dc : dc + 1],
                    )

            # v natural: v[tch] = x @ Wv^T + bv   (bf16 out)
            for tch in range(NT):
                ps = psum_acc.tile([P, D], F32, name="ps_v", tag="acc")
                for s in range(2):
                    nc.tensor.matmul(
                        ps,
                        xT8[:, tch // 4, 2 * s : 2 * s + 2,
                            (tch % 4) * P : (tch % 4 + 1) * P],
                        wvT8[:, 2 * s : 2 * s + 2, :],
                        start=(s == 0),
                        stop=(s == 1),
                        perf_mode=DR,
                    )
                nc.vector.tensor_add(vbf[:, tch, :], ps, bv_full)

        # x passthrough: out[:, 0:C] = x (DRAM->DRAM)
        for g in range(8):
            r0 = g * (T // 8)
            nc.gpsimd.dma_start(
                out=out[r0 : r0 + T // 8, 0:C], in_=x[r0 : r0 + T // 8, :]
            )

        if CFG["max_phase"] < 2:
            return
        # ---- phase 2: scores (transposed) + column-softmax ----
        for kc in range(NT):
            k0 = kc * P
            j0 = k0 // QS
            slices = [(k0, (j0 + 1) * QS - k0)]
            for j in range(j0 + 1, NQ):
                slices.append((j * QS, QS))
            ns = len(slices)

            sums = stats.tile([P, NQ], F32, name="sums", tag="sums")
            for idx, (q0, w) in enumerate(slices):
                st = psum_st.tile([P, w], F32, name="st", tag="st")
                qsj = q0 // QS
                off = q0 - qsj * QS
                for s in range(2):
                    nc.tensor.matmul(
                        st,
                        yT8[:, kc, 2 * s : 2 * s + 2, :],
                        xT8[:, qsj, 2 * s : 2 * s + 2, off : off + w],
                        start=(s == 0),
                        stop=(s == 1),
                        perf_mode=DR,
                    )
                if idx == 0:
                    # diagonal block: mask strict lower triangle (q < k)
                    nc.vector.tensor_add(st[:, 0:P], st[:, 0:P], tri)
                if CFG["max_phase"] < 3:
                    continue
                nc.scalar.activation(
                    out=e8[:, q0 // P : (q0 + w) // P, kc, :],
                    in_=st,
                    func=mybir.ActivationFunctionType.Exp,
                    bias=0.0,
                    scale=SCALE,
                    accum_out=sums[:, idx : idx + 1],
                )

            if CFG["max_phase"] < 3:
                continue
            with tc.high_priority():
                S = stats.tile([P, 1], F32, name="S", tag="S")
                nc.vector.reduce_sum(
                    out=S, in_=sums[:, 0:ns], axis=mybir.AxisListType.X
                )
                rs = stats.tile([P, 1], F32, name="rs", tag="rs")
                nc.vector.reciprocal(out=rs, in_=S)
                rs64 = stats.tile([P, 1], F32, name="rs64", tag="rs64")
                nc.vector.tensor_scalar_mul(out=rs64, in0=rs, scalar1=ESC)
                # v8[kc] = vbf[kc] * (64/S) -- normalizer folded into v
                nc.vector.tensor_scalar_mul(
                    out=v8[:, kc, :], in0=vbf[:, kc, :], scalar1=rs64
                )

        if CFG["max_phase"] < 4:
            return
        # ---- phase 3: out[qc] = (1/64) sum_j e8-pair(j, qc).T @ v8-pair(j) ----
        osb = None
        for qc in range(NT):
            ps = psum_acc.tile([P, D], F32, name="ps_o", tag="acc")
            npair = qc // 2 + 1
            for j in range(npair):
                nc.tensor.matmul(
                    ps,
                    e8[:, qc, 2 * j : 2 * j + 2, :],
                    v8[:, 2 * j : 2 * j + 2, :],
                    start=(j == 0),
                    stop=(j == npair - 1),
                    perf_mode=DR,
                )
            if qc % 2 == 0:
                osb = outsb.tile([P, 2, D], F32, name="osb")
                nc.vector.tensor_scalar_mul(
                    out=osb[:, 0, :], in0=ps, scalar1=1.0 / ESC
                )
            else:
                nc.scalar.activation(
                    out=osb[:, 1, :], in_=ps,
                    func=mybir.ActivationFunctionType.Identity, scale=1.0 / ESC,
                )
                q0 = (qc - 1) * P
                nc.sync.dma_start(
                    out=out[q0 : q0 + 2 * P, C : 2 * C].rearrange(
                        "(a p) c -> p a c", p=P
                    ),
                    in_=osb,
                )


_NC_CACHE = {}


def _get_nc(mm_dtype="f32r"):
    if mm_dtype not in _NC_CACHE:
        _NC_CACHE[mm_dtype] = build_nc(mm_dtype)
    return _NC_CACHE[mm_dtype]


def kernel(**inputs):
    from concourse.bass_utils import run_bass_kernel_spmd

    nc = _get_nc()
    x = np.asarray(inputs["x"], dtype=np.float32)
    shared = {
        name: np.ascontiguousarray(np.asarray(inputs[name], dtype=np.float32))
        for name in ("Wq", "bq", "Wk", "bk", "Wv", "bv")
    }
    in_maps = [
        {"x": np.ascontiguousarray(x[b]), **shared} for b in range(B)
    ]
    res = run_bass_kernel_spmd(nc, in_maps, core_ids=list(range(B)))
    return np.stack([res.results[b]["out"] for b in range(B)], axis=0)


# revision 19
# speedup vs baseline: 1.1496x; 1.1496x over previous
"""Trainium2 Bass kernel for nn_AttentionBlock (B=8, T=2048, C=512).

Data-parallel over batch: one batch element per NeuronCore (8 cores).

All matmuls are fp8e4 (e4m3) MatmulPerfMode.DoubleRow: the PE virtualizes
to 128x256, so a 512-deep contraction takes 2 instructions at ~174 ns
(HW-measured) vs ~323 ns for fp32r.

Score algebra: softmax is over the QUERY axis (reference quirk), which is
the FREE axis of the transposed score layout st[k, q]. Any per-k additive
term cancels between exp numerator and row-sum denominator, so with
  st[k,q] = (x Wk^T + bk)[k] . (x Wq^T + bq)[q]
the bq-dependent terms (per-k) cancel, and
  st'[k,q] = yhat[k] . x[q],   yhat = x (Wk^T Wq) + bk Wq
yields the exact same softmax. This removes the whole q-projection, the
Wq/Wk PE transposes, and one PSUM-evacuation pass.

Per-core pipeline (x: [T, C]):
  xT8[c, t]  = fp8(x^T)                      bf16 PE transpose + cast
  M8         = fp8(Wk^T Wq)  [c, c']         from NATURAL weight loads
  w_sb[c']   = bk @ Wq                       8 tiny DR matmuls
  yT8[c', t] = fp8(M^T x^T + w_sb)           "k-side" projection
  vbf[t, d]  = bf16(x @ Wv^T + bv)
  st[k, q]   = sum_c' yT8[c',k] xT8[c',q]    DR pairs over c'
  e8[k, q]   = fp8(exp(st/sqrt(D)))          causal-masked, fp32 sums S
  v8[k, :]   = fp8(vbf[k, :] * 64/S[k])      on GpSimd; 64 = fp8 range shift
  out[q, :]  = (1/64) sum_k e8[k,q] v8[k,:]  DR pairs over k-chunks
  output = concat([x, out], axis=-1)
"""

import numpy as np

import concourse.bass as bass
import concourse.mybir as mybir
import concourse.tile as tile
from concourse import bacc

B, T, C = 8, 2048, 512
D = 512                      # KEY_SIZE == VALUE_SIZE == 512
P = 128                      # partitions
NT = T // P                  # 16 t-chunks
NC4 = C // P                 # 4 contraction chunks
ND = D // P                  # 4 d-chunks
QS = 512                     # q-slice width for score matmuls
NQ = T // QS                 # 4 q-slices
SCALE = float(1.0 / np.sqrt(D))
NEG = -1.0e30
ESC = 64.0                   # fp8 range shift for attention weights

F32 = mybir.dt.float32
BF16 = mybir.dt.bfloat16
FP8 = mybir.dt.float8e4
DR = mybir.MatmulPerfMode.DoubleRow

# max_phase: 0=loads+transposes+M, 1=+projections, 2=+score MMs (no exp),
# 3=+softmax (exp/S/v8), 4=full (AV+out). For HW phase-isolation timing.
CFG = {"copy_mod": 2, "max_phase": 4}

MM_DTYPES = {"f32r": mybir.dt.float32r}  # vestigial (hw_time compat)


def build_nc(mm_dtype="f32r"):
    nc = bacc.Bacc(trn_type="TRN2", target_bir_lowering=False)

    x = nc.dram_tensor("x", [T, C], F32, kind="ExternalInput").ap()
    Wq = nc.dram_tensor("Wq", [D, C], F32, kind="ExternalInput").ap()
    bq = nc.dram_tensor("bq", [D], F32, kind="ExternalInput").ap()
    Wk = nc.dram_tensor("Wk", [D, C], F32, kind="ExternalInput").ap()
    bk = nc.dram_tensor("bk", [D], F32, kind="ExternalInput").ap()
    Wv = nc.dram_tensor("Wv", [D, C], F32, kind="ExternalInput").ap()
    bv = nc.dram_tensor("bv", [D], F32, kind="ExternalInput").ap()
    out = nc.dram_tensor("out", [T, 2 * C], F32, kind="ExternalOutput").ap()

    with tile.TileContext(nc) as tc:
        _emit(nc, tc, x, (Wq, bq), (Wk, bk), (Wv, bv), out, None)
    nc.compile()
    return nc


def _emit(nc, tc, x, wq, wk, wv, out, _mdt):
    from contextlib import ExitStack

    Wq, _bq = wq   # bq cancels in the q-axis softmax; unused
    Wk, bk = wk
    Wv, bv = wv

    _ncopy = [0]

    def copy_ps(dst, src, bias=None, eng=None):
        """PSUM->SBUF copy alternating DVE/ACT, optional per-partition bias."""
        if eng is None:
            _ncopy[0] += 1
            eng = "act" if (_ncopy[0] % CFG["copy_mod"] == 0) else "dve"
        if eng == "dve":
            if bias is None:
                nc.vector.tensor_copy(dst, src)
            else:
                nc.vector.tensor_scalar_add(out=dst, in0=src, scalar1=bias)
        else:
            if bias is None:
                nc.scalar.activation(
                    out=dst, in_=src, func=mybir.ActivationFunctionType.Identity
                )
            else:
                nc.scalar.activation(
                    out=dst, in_=src,
                    func=mybir.ActivationFunctionType.Identity, bias=bias,
                )

    with ExitStack() as ctx:
        const = ctx.enter_context(tc.tile_pool(name="const", bufs=1))
        persist = ctx.enter_context(tc.tile_pool(name="persist", bufs=1))
        stats = ctx.enter_context(tc.tile_pool(name="stats", bufs=4))
        outsb = ctx.enter_context(tc.tile_pool(name="outsb", bufs=3))
        psum_acc = ctx.enter_context(
            tc.tile_pool(name="psum_acc", bufs=4, space="PSUM")
        )
        psum_st = ctx.enter_context(
            tc.tile_pool(name="psum_st", bufs=4, space="PSUM")
        )

        # ---- constants ----
        identf = const.tile([P, P], F32, name="identf")
        nc.gpsimd.memset(identf, 0.0)
        nc.gpsimd.affine_select(
            out=identf, in_=identf, compare_op=mybir.AluOpType.not_equal,
            fill=1.0, base=0, pattern=[[-1, P]], channel_multiplier=1,
        )
        # tri[p, j] = 0 where j >= p (valid), NEG where j < p (masked)
        tri = const.tile([P, P], F32, name="tri")
        nc.gpsimd.memset(tri, 0.0)
        nc.gpsimd.affine_select(
            out=tri, in_=tri, compare_op=mybir.AluOpType.is_ge,
            fill=NEG, base=0, pattern=[[1, P]], channel_multiplier=-1,
        )
        # bv broadcast to all partitions via rank-1 fp32 matmul
        ones_f = const.tile([1, P], F32, name="ones_f")
        nc.gpsimd.memset(ones_f, 1.0)
        bv_f = const.tile([1, D], F32, name="bv_f")
        nc.gpsimd.dma_start(out=bv_f, in_=bv.unsqueeze(0))
        bv_full = const.tile([P, D], F32, name="bv_full")
        ps_bv = psum_acc.tile([P, D], F32, name="ps_bv", tag="acc")
        nc.tensor.matmul(ps_bv, ones_f, bv_f, start=True, stop=True)
        nc.vector.tensor_copy(bv_full, ps_bv)

        # bk as fp8 [128, 4, 1] for the w_sb = bk @ Wq matmuls
        bk_sb = const.tile([P, ND], F32, name="bk_sb")
        for dc in range(ND):
            nc.gpsimd.dma_start(
                out=bk_sb[:, dc : dc + 1],
                in_=bk[dc * P : (dc + 1) * P].unsqueeze(-1),
            )
        bk8 = const.tile([P, ND, 1], FP8, name="bk8")
        nc.vector.tensor_copy(bk8[:, :, 0], bk_sb)

        # ---- persistent fp8/bf16 activations ----
        xT8 = persist.tile([P, NQ, NC4, QS], FP8, name="xT8", tag="xT8")
        yT8 = persist.tile([P, NT, NC4, P], FP8, name="yT8", tag="yT8")
        e8 = persist.tile([P, NT, NT, P], FP8, name="e8", tag="e8")
        vbf = persist.tile([P, NT, D], BF16, name="vbf", tag="vbf")
        v8 = persist.tile([P, NT, D], FP8, name="v8", tag="v8")
        Wq8 = persist.tile([P, ND, C], FP8, name="Wq8", tag="Wq8")
        Wk8 = persist.tile([P, ND, C], FP8, name="Wk8", tag="Wk8")
        M8 = persist.tile([P, NC4, NC4, P], FP8, name="M8", tag="M8")
        wvT8 = persist.tile([P, NC4, D], FP8, name="wvT8", tag="wvT8")
        w_sb = const.tile([P, NC4], F32, name="w_sb")

        # zero the 8 pair-diagonal blocks of e8 that AV reads but exp never
        # writes: e8[:, 2j+1, block 2j]
        for j in range(NT // 2):
            kc = 2 * j + 1
            nc.vector.memset(e8[:, 2 * j, kc, :], 0.0)

        # ---- phase 0: loads, F32 PE transposes (cast at PSUM copy), M ----
        with tc.tile_pool(name="loads", bufs=1) as loads:
            # x loads first (the long pole): 8 batched DMAs of 2 chunks each
            xn_all = []
            for g in range(8):
                xn = loads.tile([P, 2, C], F32, name=f"xn{g}",
                                tag=f"xn{g % 4}", bufs=2)
                nc.sync.dma_start(
                    out=xn,
                    in_=x[g * 2 * P : (g + 1) * 2 * P, :].rearrange(
                        "(a p) c -> p a c", p=P
                    ),
                )
                xn_all.append(xn)

            # Wq/Wk natural loads -> fp8 via DVE cast (no transposes needed)
            for W8, W, wt in ((Wk8, Wk, "wk"), (Wq8, Wq, "wq")):
                for h in range(2):
                    wn = loads.tile([P, 2, C], F32, name=f"{wt}{h}",
                                    tag=f"{wt}{h}", bufs=1)
                    nc.sync.dma_start(
                        out=wn,
                        in_=W[h * 2 * P : (h + 1) * 2 * P, :].rearrange(
                            "(a p) c -> p a c", p=P
                        ),
                    )
                    for j in range(2):
                        nc.vector.tensor_copy(W8[:, h * 2 + j, :], wn[:, j, :])

            # M~ = Wk^T Wq  [c, c']  (DR pairs over d)
            for c1 in range(NC4):
                ps = psum_acc.tile([P, C], F32, name="ps_m", tag="acc")
                for s in range(2):
                    nc.tensor.matmul(
                        ps,
                        Wk8[:, 2 * s : 2 * s + 2, c1 * P : (c1 + 1) * P],
                        Wq8[:, 2 * s : 2 * s + 2, :],
                        start=(s == 0),
                        stop=(s == 1),
                        perf_mode=DR,
                    )
                copy_ps(M8[:, :, c1, :], ps)

            # w_sb[c'] = bk @ Wq   (tiny N=1 DR matmuls per c'-block)
            for c1 in range(NC4):
                ps = psum_acc.tile([P, 1], F32, name="ps_w", tag="acc")
                for s in range(2):
                    nc.tensor.matmul(
                        ps,
                        Wq8[:, 2 * s : 2 * s + 2, c1 * P : (c1 + 1) * P],
                        bk8[:, 2 * s : 2 * s + 2, :],
                        start=(s == 0),
                        stop=(s == 1),
                        perf_mode=DR,
                    )
                nc.vector.tensor_copy(w_sb[:, c1 : c1 + 1], ps)

            def x_group(tg):
                xsl = [xn_all[tg * 2][:, 0, :], xn_all[tg * 2][:, 1, :],
                       xn_all[tg * 2 + 1][:, 0, :], xn_all[tg * 2 + 1][:, 1, :]]
                for cc in range(NC4):
                    ps = psum_st.tile([P, D], F32, name="ps_xt", tag="st")
                    for j in range(4):
                        nc.tensor.transpose(
                            ps[:, j * P : (j + 1) * P],
                            xsl[j][:, cc * P : (cc + 1) * P],
                            identf,
                        )
                    copy_ps(xT8[:, tg, cc, :], ps)

            for tg in range(4):
                x_group(tg)

            # Wv load -> F32 PE transpose -> fp8 at copy
            wvn = []
            for h in range(2):
                wn = loads.tile([P, 2, C], F32, name=f"wvn{h}",
                                tag=f"wvn{h}", bufs=1)
                nc.sync.dma_start(
                    out=wn,
                    in_=Wv[h * 2 * P : (h + 1) * 2 * P, :].rearrange(
                        "(a p) c -> p a c", p=P
                    ),
                )
                wvn.append(wn)
            wsl = [wvn[0][:, 0, :], wvn[0][:, 1, :],
                   wvn[1][:, 0, :], wvn[1][:, 1, :]]
            for cc in range(NC4):
                ps = psum_acc.tile([P, D], F32, name="ps_wt", tag="acc")
                for dc in range(ND):
                    nc.tensor.transpose(
                        ps[:, dc * P : (dc + 1) * P],
                        wsl[dc][:, cc * P : (cc + 1) * P],
                        identf,
                    )
                copy_ps(wvT8[:, cc, :], ps, eng="act")

            if CFG["max_phase"] < 1:
                return
            # ---- phase 1: projections (fp8 DoubleRow) ----
            # yhat^T = M^T x^T + w_sb  (the bias-folded "k-side")
            for dc in range(NC4):
                for qs in range(NQ):
                    ps = psum_acc.tile([P, QS], F32, name="ps_y", tag="acc")
                    for s in range(2):
                        nc.tensor.matmul(
                            ps,
                            M8[:, dc, 2 * s : 2 * s + 2, :],
                            xT8[:, qs, 2 * s : 2 * s + 2, :],
                            start=(s == 0),
                            stop=(s == 1),
                            perf_mode=DR,
                        )
                    copy_ps(
                        yT8[:, qs * 4 : (qs + 1) * 4, dc, :], ps,
                        bias=w_sb[:, dc : dc + 1],
                    )

            # v natural: v[tch] = x @ Wv^T + bv   (bf16 out)
            for tch in range(NT):
                ps = psum_acc.tile([P, D], F32, name="ps_v", tag="acc")
                for s in range(2):
                    nc.tensor.matmul(
                        ps,
                        xT8[:, tch // 4, 2 * s : 2 * s + 2,
                            (tch % 4) * P : (tch % 4 + 1) * P],
                        wvT8[:, 2 * s : 2 * s + 2, :],
                        start=(s == 0),
                        stop=(s == 1),
                        perf_mode=DR,
                    )
                nc.vector.tensor_add(vbf[:, tch, :], ps, bv_full)

        # x passthrough: out[:, 0:C] = x (DRAM->DRAM)
        for g in range(8):
            r0 = g * (T // 8)
            nc.gpsimd.dma_start(
                out=out[r0 : r0 + T // 8, 0:C], in_=x[r0 : r0 + T // 8, :]
            )

        if CFG["max_phase"] < 2:
            return
        # ---- phase 2: scores (transposed) + column-softmax ----
        for kc in range(NT):
            k0 = kc * P
            j0 = k0 // QS
            slices = [(k0, (j0 + 1) * QS - k0)]
            for j in range(j0 + 1, NQ):
                slices.append((j * QS, QS))
            ns = len(slices)

            sums = stats.tile([P, NQ], F32, name="sums", tag="sums")
            for idx, (q0, w) in enumerate(slices):
                st = psum_st.tile([P, w], F32, name="st", tag="st")
                qsj = q0 // QS
                off = q0 - qsj * QS
                for s in range(2):
                    nc.tensor.matmul(
                        st,
                        yT8[:, kc, 2 * s : 2 * s + 2, :],
                        xT8[:, qsj, 2 * s : 2 * s + 2, off : off + w],
                        start=(s == 0),
                        stop=(s == 1),
                        perf_mode=DR,
                    )
                if idx == 0:
                    # diagonal block: mask strict lower triangle (q < k)
                    nc.vector.tensor_add(st[:, 0:P], st[:, 0:P], tri)
                if CFG["max_phase"] < 3:
                    continue
                nc.scalar.activation(
                    out=e8[:, q0 // P : (q0 + w) // P, kc, :],
                    in_=st,
                    func=mybir.ActivationFunctionType.Exp,
                    bias=0.0,
                    scale=SCALE,
                    accum_out=sums[:, idx : idx + 1],
                )

            if CFG["max_phase"] < 3:
                continue
            with tc.high_priority():
                S = stats.tile([P, 1], F32, name="S", tag="S")
                nc.vector.reduce_sum(
                    out=S, in_=sums[:, 0:ns], axis=mybir.AxisListType.X
                )
                rs = stats.tile([P, 1], F32, name="rs", tag="rs")
                nc.vector.reciprocal(out=rs, in_=S)
                rs64 = stats.tile([P, 1], F32, name="rs64", tag="rs64")
                nc.vector.tensor_scalar_mul(out=rs64, in0=rs, scalar1=ESC)
                # v8[kc] = vbf[kc] * (64/S) -- normalizer folded into v
                nc.vector.tensor_scalar_mul(
                    out=v8[:, kc, :], in0=vbf[:, kc, :], scalar1=rs64
                )

        if CFG["max_phase"] < 4:
            return
        # ---- phase 3: out[qc] = (1/64) sum_j e8-pair(j, qc).T @ v8-pair(j) ----
        osb = None
        for qc in range(NT):
            ps = psum_acc.tile([P, D], F32, name="ps_o", tag="acc")
            npair = qc // 2 + 1
            for j in range(npair):
                nc.tensor.matmul(
                    ps,
                    e8[:, qc, 2 * j : 2 * j + 2, :],
                    v8[:, 2 * j : 2 * j + 2, :],
                    start=(j == 0),
                    stop=(j == npair - 1),
                    perf_mode=DR,
                )
            if qc % 2 == 0:
                osb = outsb.tile([P, 2, D], F32, name="osb")
                nc.vector.tensor_scalar_mul(
                    out=osb[:, 0, :], in0=ps, scalar1=1.0 / ESC
                )
            else:
                nc.scalar.activation(
                    out=osb[:, 1, :], in_=ps,
                    func=mybir.ActivationFunctionType.Identity, scale=1.0 / ESC,
                )
                q0 = (qc - 1) * P
                nc.sync.dma_start(
                    out=out[q0 : q0 + 2 * P, C : 2 * C].rearrange(
                        "(a p) c -> p a c", p=P
                    ),
                    in_=osb,
                )


_NC_CACHE = {}


def _get_nc(mm_dtype="f32r"):
    if mm_dtype not in _NC_CACHE:
        _NC_CACHE[mm_dtype] = build_nc(mm_dtype)
    return _NC_CACHE[mm_dtype]


def kernel(**inputs):
    from concourse.bass_utils import run_bass_kernel_spmd

    nc = _get_nc()
    x = np.asarray(inputs["x"], dtype=np.float32)
    shared = {
        name: np.ascontiguousarray(np.asarray(inputs[name], dtype=np.float32))
        for name in ("Wq", "bq", "Wk", "bk", "Wv", "bv")
    }
    in_maps = [
        {"x": np.ascontiguousarray(x[b]), **shared} for b in range(B)
    ]
    res = run_bass_kernel_spmd(nc, in_maps, core_ids=list(range(B)))
    return np.stack([res.results[b]["out"] for b in range(B)], axis=0)
